# revision 22
# baseline (speedup 1.0000x reference)
"""Trainium2 Bass kernel for ContrastiveMSELoss.

Reference computes, over all N^2 pairs (diagonal masked to 0):
    mse_ij  = (|x_i|^2 + |x_j|^2 - 2 x_i.x_j) / D
    sign_ij = +1 if class_i == class_j else -1
    loss    = mean_ij(sign_ij * mse_ij) + BETA

Using sum_{i,j in c} x_i.x_j = |M_c|^2 with M_c = sum_{i in c} x_i, the
loss collapses to class-bucketed first/second moments (O(N*D) work,
memory-bound -- no N x N gram matrix needed):

    T_same = sum_c (2 n_c SQ_c - 2 |M_c|^2) / D      (diag terms are 0)
    T_all  = (2 N SQ - 2 |M|^2) / D
    loss   = (2 T_same - T_all) / N^2 + BETA

Sharding: rows are split across 8 cores.  Each core packs [X | X^2] into a
bf16 [128, 512] rhs per 128-row chunk and one-hot class rows into the lhsT,
so a single accumulating matmul chain produces the partial per-class sums
M_c (cols 0..D-1) and per-dim squared sums (cols D..2D-1).  The host
combines the 8 partial [40, 512] outputs in float64.
"""

import numpy as np

import concourse.bacc as bacc
import concourse.bass as bass
import concourse.tile as tile
from concourse import mybir
from concourse.bass_utils import run_bass_kernel_spmd

N, D = 8192, 256
N_CORES = 8
ROWS = N // N_CORES          # 1024 rows per core
P = 128                      # partitions
CHUNKS = ROWS // P           # 8 chunks of 128 rows
NCLS = 40
BETA = 1.0
HALF = CHUNKS // 2           # chunks per pipeline half

_CACHE = {}


def _bcast(ap, pos, count):
    """Insert a zero-stride dim of size `count` at free-dim position `pos`."""
    pattern = [list(p) for p in ap.ap]
    pattern.insert(pos, [0, count])
    return bass.AP(tensor=ap.tensor, offset=ap.offset, ap=pattern)


def _build_bass():
    nc = bacc.Bacc(
        "TRN2",
        target_bir_lowering=False,
        debug=False,
        enable_asserts=True,
        num_devices=N_CORES,
    )
    x = nc.dram_tensor("x", [ROWS, D], mybir.dt.float32, kind="ExternalInput")
    # combo[p, :NCLS] = iota row 0..39 (host constant); combo[p, NCLS + k] =
    # class id (as f32) of shard row k*128 + p.  One tensor = one DMA issue.
    combo = nc.dram_tensor(
        "combo", [P, NCLS + CHUNKS], mybir.dt.float32, kind="ExternalInput"
    )
    # stats[c, :D] = sum of rows with class c; stats[c, D] = sum of |x_i|^2
    stats = nc.dram_tensor(
        "stats", [NCLS, D + 1], mybir.dt.float32, kind="ExternalOutput"
    )

    with tile.TileContext(nc) as tc:
        with (
            tc.tile_pool(name="work", bufs=1) as work,
            tc.tile_pool(name="psum", bufs=1, space="PSUM") as psum_pool,
        ):
            # raw f32 input: one 128-row chunk per DMA so each lands on its
            # own HW queue; sync issues even chunks (x0 first), scalar
            # (whose stream starts with the ACT table load) odd chunks.
            # The iota/cls combo rides second on sync.
            xf = work.tile([P, CHUNKS, D], mybir.dt.float32, tag="xf")
            combo_sb = work.tile([P, NCLS + CHUNKS], mybir.dt.float32, tag="combo_sb")
            sync_chunks = [0, 2, 4, 6]
            scalar_chunks = [1, 3, 5, 7]
            nc.sync.dma_start(out=xf[:, 0, :], in_=x[0:P, :])
            nc.sync.dma_start(out=combo_sb, in_=combo[:, :])
            for k in sync_chunks[1:]:
                nc.sync.dma_start(out=xf[:, k, :], in_=x[k * P : (k + 1) * P, :])
            for k in scalar_chunks:
                nc.scalar.dma_start(out=xf[:, k, :], in_=x[k * P : (k + 1) * P, :])
            iota_sb = combo_sb[:, :NCLS]
            cls_sb = combo_sb[:, NCLS:]

            # bf16 matmul operands: [X | row-sum(X^2)] and one-hot classes
            xb = work.tile([P, CHUNKS, D + 1], mybir.dt.bfloat16, tag="xb")
            sq = work.tile([P, CHUNKS], mybir.dt.float32, tag="sq")
            sqscr = work.tile([P, CHUNKS, D], mybir.dt.bfloat16, tag="sqscr")
            oh = work.tile([P, CHUNKS, NCLS], mybir.dt.bfloat16, tag="oh")
            acc = psum_pool.tile([NCLS, D + 1], mybir.dt.float32, tag="acc")

            # one-hot: oh[p, k, c] = (cls[p, k] == c), one broadcast op,
            # emitted first so the weights are ready before the matmuls
            nc.vector.tensor_tensor(
                out=oh[:, :, :],
                in0=_bcast(cls_sb, 2, NCLS),
                in1=_bcast(iota_sb, 1, CHUNKS),
                op=mybir.AluOpType.is_equal,
            )
            for k in range(CHUNKS):
                # cast X -> bf16 (DVE); |row|^2 via ACT Square + accum_out
                # (the squared values themselves never reach the PE)
                nc.vector.tensor_copy(xb[:, k, :D], xf[:, k, :])
                nc.scalar.activation(
                    out=sqscr[:, k, :],
                    in_=xf[:, k, :],
                    func=mybir.ActivationFunctionType.Square,
                    accum_out=sq[:, k : k + 1],
                )
                nc.vector.tensor_copy(xb[:, k, D : D + 1], sq[:, k : k + 1])
                nc.tensor.matmul(
                    acc,
                    oh[:, k, :],
                    xb[:, k, :],
                    start=(k == 0),
                    stop=(k == CHUNKS - 1),
                )

            out_sb = work.tile([NCLS, D + 1], mybir.dt.float32, tag="out_sb")
            nc.vector.tensor_copy(out_sb, acc)
            nc.sync.dma_start(out=stats[:, :], in_=out_sb)

    return nc


def _get_nc():
    if "nc" not in _CACHE:
        nc = _build_bass()
        nc.finalize()
        _CACHE["nc"] = nc
    return _CACHE["nc"]


_IOTA = np.broadcast_to(np.arange(NCLS, dtype=np.float32), (P, NCLS))


def run_device(output, classes, **spmd_kwargs):
    """Run the per-core Bass kernel; returns (list of per-core stats, results)."""
    x = np.ascontiguousarray(np.asarray(output), dtype=np.float32)
    cls_f = np.asarray(classes).astype(np.float32)
    in_maps = []
    for s in range(N_CORES):
        xs = x[s * ROWS : (s + 1) * ROWS]
        cs = cls_f[s * ROWS : (s + 1) * ROWS]
        # combo[:, :NCLS] = iota; combo[:, NCLS + k] = class of row k*128+p
        combo = np.concatenate([_IOTA, cs.reshape(CHUNKS, P).T], axis=1)
        in_maps.append({"x": xs, "combo": np.ascontiguousarray(combo)})
    res = run_bass_kernel_spmd(
        _get_nc(), in_maps, core_ids=list(range(N_CORES)), **spmd_kwargs
    )
    stats = [res.results[s]["stats"] for s in range(N_CORES)]
    return stats, res


def _combine(stats, classes):
    """Combine per-core partial class stats into the scalar loss (float64)."""
    tot = np.sum(np.asarray(stats, dtype=np.float64), axis=0)  # [NCLS, D+1]
    M_c = tot[:, :D]                                           # class sums
    SQ_c = tot[:, D]                                           # class |x|^2 sums
    n_c = np.bincount(np.asarray(classes).astype(np.int64), minlength=NCLS).astype(
        np.float64
    )
    SQ = SQ_c.sum()
    M = M_c.sum(axis=0)
    T_same = (2.0 * (n_c * SQ_c).sum() - 2.0 * (M_c * M_c).sum()) / D
    T_all = (2.0 * N * SQ - 2.0 * (M @ M)) / D
    loss = (2.0 * T_same - T_all) / (float(N) * float(N)) + BETA
    return np.float32(loss)


def kernel(output, classes):
    stats, _ = run_device(output, classes)
    return _combine(stats, classes)


# revision 23
# speedup vs baseline: 1.1142x; 1.1142x over previous
"""Trainium2 Bass kernel for ContrastiveMSELoss.

Reference computes, over all N^2 pairs (diagonal masked to 0):
    mse_ij  = (|x_i|^2 + |x_j|^2 - 2 x_i.x_j) / D
    sign_ij = +1 if class_i == class_j else -1
    loss    = mean_ij(sign_ij * mse_ij) + BETA

Using sum_{i,j in c} x_i.x_j = |M_c|^2 with M_c = sum_{i in c} x_i, the
loss collapses to class-bucketed first/second moments (O(N*D) work,
memory-bound -- no N x N gram matrix needed):

    T_same = sum_c (2 n_c SQ_c - 2 |M_c|^2) / D      (diag terms are 0)
    T_all  = (2 N SQ - 2 |M|^2) / D
    loss   = (2 T_same - T_all) / N^2 + BETA

Sharding: rows are split across 8 cores.  Each core packs [X | X^2] into a
bf16 [128, 512] rhs per 128-row chunk and one-hot class rows into the lhsT,
so a single accumulating matmul chain produces the partial per-class sums
M_c (cols 0..D-1) and per-dim squared sums (cols D..2D-1).  The host
combines the 8 partial [40, 512] outputs in float64.
"""

import numpy as np

import concourse.bacc as bacc
import concourse.bass as bass
import concourse.tile as tile
from concourse import mybir
from concourse.bass_utils import run_bass_kernel_spmd

N, D = 8192, 256
N_CORES = 8
ROWS = N // N_CORES          # 1024 rows per core
P = 128                      # partitions
CHUNKS = ROWS // P           # 8 chunks of 128 rows
NCLS = 40
BETA = 1.0
HALF = CHUNKS // 2           # chunks per pipeline half

_CACHE = {}


def _bcast(ap, pos, count):
    """Insert a zero-stride dim of size `count` at free-dim position `pos`."""
    pattern = [list(p) for p in ap.ap]
    pattern.insert(pos, [0, count])
    return bass.AP(tensor=ap.tensor, offset=ap.offset, ap=pattern)


def _build_bass():
    nc = bacc.Bacc(
        "TRN2",
        target_bir_lowering=False,
        debug=False,
        enable_asserts=True,
        num_devices=N_CORES,
    )
    x = nc.dram_tensor("x", [ROWS, D], mybir.dt.float32, kind="ExternalInput")
    # combo[p, :NCLS] = iota row 0..39 (host constant); combo[p, NCLS + k] =
    # class id (as f32) of shard row k*128 + p.  One tensor = one DMA issue.
    combo = nc.dram_tensor(
        "combo", [P, NCLS + CHUNKS], mybir.dt.float32, kind="ExternalInput"
    )
    # stats[c, :D] = sum of rows with class c; stats[c, D] = sum of |x_i|^2
    stats = nc.dram_tensor(
        "stats", [NCLS, D + 1], mybir.dt.float32, kind="ExternalOutput"
    )

    with tile.TileContext(nc) as tc:
        with (
            tc.tile_pool(name="work", bufs=1) as work,
            tc.tile_pool(name="psum", bufs=1, space="PSUM") as psum_pool,
        ):
            # raw f32 input: one 128-row chunk per DMA so each lands on its
            # own HW queue; sync issues even chunks (x0 first), scalar
            # (whose stream starts with the ACT table load) odd chunks.
            # The iota/cls combo rides second on sync.
            xf = work.tile([P, CHUNKS, D], mybir.dt.float32, tag="xf")
            combo_sb = work.tile([P, NCLS + CHUNKS], mybir.dt.float32, tag="combo_sb")
            sync_chunks = [0, 2, 4, 6]
            scalar_chunks = [1, 3, 5, 7]
            nc.sync.dma_start(out=xf[:, 0, :], in_=x[0:P, :])
            nc.sync.dma_start(out=combo_sb, in_=combo[:, :])
            for k in sync_chunks[1:]:
                nc.sync.dma_start(out=xf[:, k, :], in_=x[k * P : (k + 1) * P, :])
            for k in scalar_chunks:
                nc.scalar.dma_start(out=xf[:, k, :], in_=x[k * P : (k + 1) * P, :])
            iota_sb = combo_sb[:, :NCLS]
            cls_sb = combo_sb[:, NCLS:]

            # bf16 matmul operands: [X | X^2] and one-hot classes
            xb = work.tile([P, CHUNKS, 2 * D], mybir.dt.bfloat16, tag="xb")
            oh = work.tile([P, CHUNKS, NCLS], mybir.dt.bfloat16, tag="oh")
            acc = psum_pool.tile([NCLS, 2 * D], mybir.dt.float32, tag="acc")

            # one-hot: oh[p, k, c] = (cls[p, k] == c), one broadcast op,
            # emitted first so the weights are ready before the matmuls
            nc.vector.tensor_tensor(
                out=oh[:, :, :],
                in0=_bcast(cls_sb, 2, NCLS),
                in1=_bcast(iota_sb, 1, CHUNKS),
                op=mybir.AluOpType.is_equal,
            )
            for k in range(CHUNKS):
                # cast X -> bf16 (DVE); X^2 -> bf16 alternating between DVE
                # (tensor_mul) and ACT (Square) so neither engine paces the
                # matmul chain alone
                nc.vector.tensor_copy(xb[:, k, :D], xf[:, k, :])
                if k % 2 == 0:
                    nc.vector.tensor_mul(xb[:, k, D:], xf[:, k, :], xf[:, k, :])
                else:
                    nc.scalar.activation(
                        out=xb[:, k, D:],
                        in_=xf[:, k, :],
                        func=mybir.ActivationFunctionType.Square,
                    )
                nc.tensor.matmul(
                    acc,
                    oh[:, k, :],
                    xb[:, k, :],
                    start=(k == 0),
                    stop=(k == CHUNKS - 1),
                )

            # fold the per-dim x^2 sums to a single column on-chip so the
            # result DMA is half the size
            out_sb = work.tile([NCLS, D + 1], mybir.dt.float32, tag="out_sb")
            nc.vector.tensor_copy(out_sb[:, :D], acc[:, :D])
            nc.vector.reduce_sum(
                out=out_sb[:, D : D + 1], in_=acc[:, D:], axis=mybir.AxisListType.X
            )
            nc.sync.dma_start(out=stats[:, :], in_=out_sb)

    return nc


def _get_nc():
    if "nc" not in _CACHE:
        nc = _build_bass()
        nc.finalize()
        _CACHE["nc"] = nc
    return _CACHE["nc"]


_IOTA = np.broadcast_to(np.arange(NCLS, dtype=np.float32), (P, NCLS))


def run_device(output, classes, **spmd_kwargs):
    """Run the per-core Bass kernel; returns (list of per-core stats, results)."""
    x = np.ascontiguousarray(np.asarray(output), dtype=np.float32)
    cls_f = np.asarray(classes).astype(np.float32)
    in_maps = []
    for s in range(N_CORES):
        xs = x[s * ROWS : (s + 1) * ROWS]
        cs = cls_f[s * ROWS : (s + 1) * ROWS]
        # combo[:, :NCLS] = iota; combo[:, NCLS + k] = class of row k*128+p
        combo = np.concatenate([_IOTA, cs.reshape(CHUNKS, P).T], axis=1)
        in_maps.append({"x": xs, "combo": np.ascontiguousarray(combo)})
    res = run_bass_kernel_spmd(
        _get_nc(), in_maps, core_ids=list(range(N_CORES)), **spmd_kwargs
    )
    stats = [res.results[s]["stats"] for s in range(N_CORES)]
    return stats, res


def _combine(stats, classes):
    """Combine per-core partial class stats into the scalar loss (float64)."""
    tot = np.sum(np.asarray(stats, dtype=np.float64), axis=0)  # [NCLS, D+1]
    M_c = tot[:, :D]                                           # class sums
    SQ_c = tot[:, D]                                           # class |x|^2 sums
    n_c = np.bincount(np.asarray(classes).astype(np.int64), minlength=NCLS).astype(
        np.float64
    )
    SQ = SQ_c.sum()
    M = M_c.sum(axis=0)
    T_same = (2.0 * (n_c * SQ_c).sum() - 2.0 * (M_c * M_c).sum()) / D
    T_all = (2.0 * N * SQ - 2.0 * (M @ M)) / D
    loss = (2.0 * T_same - T_all) / (float(N) * float(N)) + BETA
    return np.float32(loss)


def kernel(output, classes):
    stats, _ = run_device(output, classes)
    return _combine(stats, classes)


# revision 24
# speedup vs baseline: 1.1351x; 1.0187x over previous
"""Trainium2 Bass kernel for ContrastiveMSELoss.

Reference computes, over all N^2 pairs (diagonal masked to 0):
    mse_ij  = (|x_i|^2 + |x_j|^2 - 2 x_i.x_j) / D
    sign_ij = +1 if class_i == class_j else -1
    loss    = mean_ij(sign_ij * mse_ij) + BETA

Using sum_{i,j in c} x_i.x_j = |M_c|^2 with M_c = sum_{i in c} x_i, the
loss collapses to class-bucketed first/second moments (O(N*D) work,
memory-bound -- no N x N gram matrix needed):

    T_same = sum_c (2 n_c SQ_c - 2 |M_c|^2) / D      (diag terms are 0)
    T_all  = (2 N SQ - 2 |M|^2) / D
    loss   = (2 T_same - T_all) / N^2 + BETA

Sharding: rows are split across 8 cores.  Each core packs [X | X^2] into a
bf16 [128, 512] rhs per 128-row chunk and one-hot class rows into the lhsT,
so a single accumulating matmul chain produces the partial per-class sums
M_c and per-dim squared sums; the squared sums are folded to one column
on-chip and the host combines the 8 partial [40, 257] outputs in float64.
"""

import numpy as np

import concourse.bacc as bacc
import concourse.bass as bass
import concourse.tile as tile
from concourse import mybir
from concourse.bass_utils import run_bass_kernel_spmd

N, D = 8192, 256
N_CORES = 8
ROWS = N // N_CORES          # 1024 rows per core
P = 128                      # partitions
CHUNKS = ROWS // P           # 8 chunks of 128 rows
NCLS = 40
BETA = 1.0
HALF = CHUNKS // 2           # chunks per pipeline half

_CACHE = {}


def _bcast(ap, pos, count):
    """Insert a zero-stride dim of size `count` at free-dim position `pos`."""
    pattern = [list(p) for p in ap.ap]
    pattern.insert(pos, [0, count])
    return bass.AP(tensor=ap.tensor, offset=ap.offset, ap=pattern)


def _build_bass():
    nc = bacc.Bacc(
        "TRN2",
        target_bir_lowering=False,
        debug=False,
        enable_asserts=True,
        num_devices=N_CORES,
    )
    x = nc.dram_tensor("x", [ROWS, D], mybir.dt.float32, kind="ExternalInput")
    # combo[p, :NCLS] = iota row 0..39 (host constant); combo[p, NCLS + k] =
    # class id (as f32) of shard row k*128 + p.  One tensor = one DMA issue.
    combo = nc.dram_tensor(
        "combo", [P, NCLS + CHUNKS], mybir.dt.float32, kind="ExternalInput"
    )
    # stats[c, :D] = sum of rows with class c; stats[c, D] = sum of |x_i|^2
    stats = nc.dram_tensor(
        "stats", [NCLS, D + 1], mybir.dt.float32, kind="ExternalOutput"
    )

    with tile.TileContext(nc) as tc:
        with (
            tc.tile_pool(name="work", bufs=1) as work,
            tc.tile_pool(name="psum", bufs=1, space="PSUM") as psum_pool,
        ):
            # raw f32 input: one 128-row chunk per DMA so each lands on its
            # own HW queue; sync issues even chunks (x0 first), scalar
            # (whose stream starts with the ACT table load) odd chunks.
            # The iota/cls combo rides second on sync.
            xf = work.tile([P, CHUNKS, D], mybir.dt.float32, tag="xf")
            combo_sb = work.tile([P, NCLS + CHUNKS], mybir.dt.float32, tag="combo_sb")
            sync_chunks = [0, 2, 4, 6]
            scalar_chunks = [1, 3, 5, 7]
            nc.sync.dma_start(out=xf[:, 0, :], in_=x[0:P, :])
            nc.sync.dma_start(out=combo_sb, in_=combo[:, :])
            for k in sync_chunks[1:]:
                nc.sync.dma_start(out=xf[:, k, :], in_=x[k * P : (k + 1) * P, :])
            for k in scalar_chunks:
                nc.scalar.dma_start(out=xf[:, k, :], in_=x[k * P : (k + 1) * P, :])
            iota_sb = combo_sb[:, :NCLS]
            cls_sb = combo_sb[:, NCLS:]

            # bf16 matmul operands: [X | X^2] and one-hot classes
            xb = work.tile([P, CHUNKS, 2 * D], mybir.dt.bfloat16, tag="xb")
            oh = work.tile([P, CHUNKS, NCLS], mybir.dt.bfloat16, tag="oh")
            acc = psum_pool.tile([NCLS, 2 * D], mybir.dt.float32, tag="acc")

            # one-hot: oh[p, k, c] = (cls[p, k] == c), one broadcast op,
            # emitted first so the weights are ready before the matmuls
            nc.vector.tensor_tensor(
                out=oh[:, :, :],
                in0=_bcast(cls_sb, 2, NCLS),
                in1=_bcast(iota_sb, 1, CHUNKS),
                op=mybir.AluOpType.is_equal,
            )
            for k in range(CHUNKS):
                # cast X -> bf16 (DVE); X^2 -> bf16 alternating between DVE
                # (tensor_mul) and ACT (Square) so neither engine paces the
                # matmul chain alone
                nc.vector.tensor_copy(xb[:, k, :D], xf[:, k, :])
                if k % 2 == 0:
                    nc.vector.tensor_mul(xb[:, k, D:], xf[:, k, :], xf[:, k, :])
                else:
                    nc.scalar.activation(
                        out=xb[:, k, D:],
                        in_=xf[:, k, :],
                        func=mybir.ActivationFunctionType.Square,
                    )
                nc.tensor.matmul(
                    acc,
                    oh[:, k, :],
                    xb[:, k, :],
                    start=(k == 0),
                    stop=(k == CHUNKS - 1),
                )

            # fold the per-dim x^2 sums to a single column on-chip so the
            # result DMA is half the size
            out_sb = work.tile([NCLS, D + 1], mybir.dt.float32, tag="out_sb")
            nc.vector.tensor_copy(out_sb[:, :D], acc[:, :D])
            nc.vector.reduce_sum(
                out=out_sb[:, D : D + 1], in_=acc[:, D:], axis=mybir.AxisListType.X
            )
            nc.sync.dma_start(out=stats[:, :], in_=out_sb)

    return nc


def _get_nc():
    if "nc" not in _CACHE:
        nc = _build_bass()
        nc.finalize()
        _CACHE["nc"] = nc
    return _CACHE["nc"]


_IOTA = np.broadcast_to(np.arange(NCLS, dtype=np.float32), (P, NCLS))


def run_device(output, classes, **spmd_kwargs):
    """Run the per-core Bass kernel; returns (list of per-core stats, results)."""
    x = np.ascontiguousarray(np.asarray(output), dtype=np.float32)
    cls_f = np.asarray(classes).astype(np.float32)
    in_maps = []
    for s in range(N_CORES):
        xs = x[s * ROWS : (s + 1) * ROWS]
        cs = cls_f[s * ROWS : (s + 1) * ROWS]
        # combo[:, :NCLS] = iota; combo[:, NCLS + k] = class of row k*128+p
        combo = np.concatenate([_IOTA, cs.reshape(CHUNKS, P).T], axis=1)
        in_maps.append({"x": xs, "combo": np.ascontiguousarray(combo)})
    res = run_bass_kernel_spmd(
        _get_nc(), in_maps, core_ids=list(range(N_CORES)), **spmd_kwargs
    )
    stats = [res.results[s]["stats"] for s in range(N_CORES)]
    return stats, res


def _combine(stats, classes):
    """Combine per-core partial class stats into the scalar loss (float64)."""
    tot = np.sum(np.asarray(stats, dtype=np.float64), axis=0)  # [NCLS, D+1]
    M_c = tot[:, :D]                                           # class sums
    SQ_c = tot[:, D]                                           # class |x|^2 sums
    n_c = np.bincount(np.asarray(classes).astype(np.int64), minlength=NCLS).astype(
        np.float64
    )
    SQ = SQ_c.sum()
    M = M_c.sum(axis=0)
    T_same = (2.0 * (n_c * SQ_c).sum() - 2.0 * (M_c * M_c).sum()) / D
    T_all = (2.0 * N * SQ - 2.0 * (M @ M)) / D
    loss = (2.0 * T_same - T_all) / (float(N) * float(N)) + BETA
    return np.float32(loss)


def kernel(output, classes):
    stats, _ = run_device(output, classes)
    return _combine(stats, classes)
